# revision 41
# baseline (speedup 1.0000x reference)
"""Trainium2 Bass kernel for ASSRN sparse-attention SR head (self-contained).

kernel(**inputs) takes the FULL numpy inputs (named as in setup_inputs) and
returns the FULL [B, Q, 1] float32 output. The flattened query dim Q is
sharded over 8 NeuronCores; the MLP weights, the unfolded LR feature table
and a 2x2-pixel-packed padded HR reference table are replicated.

Algebraic simplification: the reference softmax is over a singleton axis so
it is exactly 1.0 and the whole query/k-branch cancels:
    out = MLP_q(concat_off(value*wv + ref_s*wr)) + bilinear(x).

Device dataflow per (128-query tile, offset):
  - one indirect-DMA gather of the LR unfolded-feature row and one of the
    packed 2x2 HR pixel block per query (1 index per partition),
  - bilinear blend of the 4 HR pixels with per-partition scalar weights
    (ACT muls + DVE fused multiply-adds + GpSimd final add),
  - PE transposes query-major -> feature-major, copy-back to fp32r/bf16,
  - v/r MLPs (580->256 relu ->576); elementwise gating fused on DVE as
    (psum + b2) * value,
  - pred_v + pred_r and the offset concat are absorbed into the final MLP's
    PSUM accumulation; + host-precomputed bilinear(x).

BF16_L1 puts the first-layer path (tables, gathers, blend, transposes,
copy-backs, W1) in bf16: halves gather DMA and doubles DVE mode rates.
mm2/mm3 stay fp32r off the f32 PSUM.
"""

import sys
import numpy as np

sys.path.insert(0, "/opt/trn_rl_repo")

import ml_dtypes
import concourse.bass as bass
import concourse.bacc as bacc
import concourse.mybir as mybir
import concourse.tile as tile
from concourse.bass_utils import run_bass_kernel_spmd
from concourse.masks import make_identity
from contextlib import ExitStack

F32 = mybir.dt.float32
F32R = mybir.dt.float32r
BF16 = mybir.dt.bfloat16
I32 = mybir.dt.int32

BF16_L1 = True
POOL_ADD = True
SUM_PRED = False    # sum pred_v+pred_r on DVE instead of doubling mm3
QW1_BF = True       # final-MLP W1 + pred tiles in bf16
FP8_MM23 = True     # mm2 + mm3 in fp8e4m3 DoubleRow (weights x64, dequant
                    # folded into the tail relu's scale)
FP8_MM1 = True      # mm1 in fp8 DoubleRow too: act tiles fp8 (conversion
                    # rides the copy-backs), w1 x64 fp8, dequant in h-relu
F8SC = 64.0         # fp8 weight scale
CPB_DMA = False     # PSUM->SBUF copy-backs via DMA (unsupported: DMA
                    # cannot read PSUM)
BLEND_POOL = False  # GpSimd STT fails walrus codegen -> keep on DVE
BLEND_DIAG = False  # diag-weighted transposes measured +1.5% rel err on HW
                    # (transpose-mode multiply precision); keep blends on
                    # ACT/DVE
BLEND_POOL2 = False  # move 2nd blend STT to GpSimd (unsupported on Pool)
COPY_MODE = 1       # 0: alternate DVE/ACT, 1: all DVE, 2: DVE 4 of 5
FEAT4 = True        # single 2x2-block feat gather (host-verified exact)
RELU_DVE = False    # hidden-relu on DVE instead of ACT

B, C, H, W = 2, 64, 64, 64
HR, WR = 128, 128
Q = HR * WR
C9 = C * 9          # 576
HID = 256
NCORES = 8
QSH = Q // NCORES   # 2048 queries per core per batch
NT = 32             # 128-query tiles per core
CHQ = 512
NCH = 4096 // CHQ   # query chunks per core (2 batches x 2048)


def set_chq(v):
    global CHQ, NCH
    CHQ = v
    NCH = 4096 // v
PADHR = HR + 4      # 132
KW1 = [128, 128, 128, 128, 68]
JW = [128, 128, 128, 128, 64]
OFFS = [(-1, -1), (-1, 1), (1, -1), (1, 1)]

_f = np.float32


# ---------------------------------------------------------------- host prep
def _unfold_table(feature):
    # FU[b*4096 + y*64 + x, c*9 + (i*3+j)] = feature[b, c, y+i-1, x+j-1] (0-pad)
    fp = np.pad(feature, ((0, 0), (0, 0), (1, 1), (1, 1)))
    stk = np.empty((B, C, 9, H, W), np.float32)
    for i in range(3):
        for j in range(3):
            stk[:, :, i * 3 + j] = fp[:, :, i:i + H, j:j + W]
    return np.ascontiguousarray(stk.transpose(0, 3, 4, 1, 2)).reshape(B * H * W, C9)


def _ref_table4(ref):
    # RF4[b*132*132 + y*132 + x] = [p(y,x), p(y,x+1), p(y+1,x), p(y+1,x+1)]
    # of the 2-pixel-padded HR image (zeros outside).
    rp = np.zeros((B, PADHR, PADHR, C9), np.float32)
    rp[:, 2:2 + HR, 2:2 + WR, :] = ref.transpose(0, 2, 3, 1)
    rf4 = np.zeros((B, PADHR, PADHR, 4, C9), np.float32)
    rf4[:, :PADHR - 1, :PADHR - 1, 0] = rp[:, :PADHR - 1, :PADHR - 1]
    rf4[:, :PADHR - 1, :PADHR - 1, 1] = rp[:, :PADHR - 1, 1:]
    rf4[:, :PADHR - 1, :PADHR - 1, 2] = rp[:, 1:, :PADHR - 1]
    rf4[:, :PADHR - 1, :PADHR - 1, 3] = rp[:, 1:, 1:]
    return rf4.reshape(B * PADHR * PADHR, 4 * C9)


def _feat_table4(feature):
    # 9-variant 2x2-block table: variant (rs, cs) in {0,1,2}^2 packs
    # [f(k,l), f(k,l+cs), f(k+rs,l), f(k+rs,l+cs)] (min-clamped padding for
    # rows/cols that are never indexed). The host picks the variant per query
    # from the actual offset-pixel steps, so the gather is exact.
    fu = _unfold_table(feature).reshape(B, H, W, C9)
    out = np.empty((B, 9, H, W, 4 * C9), np.float32)
    for rs in range(3):
        kk = np.minimum(np.arange(H) + rs, H - 1)
        for cs in range(3):
            ll = np.minimum(np.arange(W) + cs, W - 1)
            v = rs * 3 + cs
            out[:, v, :, :, 0:C9] = fu
            out[:, v, :, :, C9:2 * C9] = fu[:, :, ll]
            out[:, v, :, :, 2 * C9:3 * C9] = fu[:, kk]
            out[:, v, :, :, 3 * C9:4 * C9] = fu[:, kk][:, :, ll]
    return out.reshape(B * 9 * H * W, 4 * C9)


def _nearest(g, n):
    f = ((g + _f(1.0)) * _f(n) - _f(1.0)) * _f(0.5)
    return np.clip(np.round(f), 0, n - 1).astype(np.int32)


def _tables(coord, cell, x):
    """Index/weight/extra tables, mirroring the reference f32 math exactly."""
    EPS = _f(1e-6)
    one = _f(1.0)
    lo, hi = _f(-1 + 1e-6), _f(1 - 1e-6)
    cx = coord[:, :, 0]
    cy = coord[:, :, 1]
    s0 = cell[:, 0, 0][:, None]
    s1 = cell[:, 0, 1][:, None]
    rx = (one - s0) / _f(H - 1)
    ry = (one - s1) / _f(W - 1)
    sc0 = cell[:, :, 0] * _f(H)
    sc1 = cell[:, :, 1] * _f(W)

    ridx = np.empty((B, 4, Q), np.int32)
    fidx = np.empty((B, 4, Q), np.int32)
    ihs = np.empty((B, 4, Q), np.int32)
    iws = np.empty((B, 4, Q), np.int32)
    wts = np.empty((B, 4, 4, Q), np.float32)
    extra = np.empty((B, 4, 4, Q), np.float32)

    for o, (vx, vy) in enumerate(OFFS):
        ch = np.clip(cx + _f(vx) * rx + EPS, lo, hi)
        cw = np.clip(cy + _f(vy) * ry + EPS, lo, hi)
        ih = _nearest(ch, H)
        iw = _nearest(cw, W)
        ihs[:, o] = ih
        iws[:, o] = iw
        fidx[:, o] = (np.arange(B, dtype=np.int32)[:, None] * (H * W)
                      + ih * W + iw)
        ckx = (_f(2.0) * ih.astype(np.float32) + one) / _f(H) - one
        cky = (_f(2.0) * iw.astype(np.float32) + one) / _f(W) - one
        extra[:, o, 0] = (cx - ckx) * _f(H)
        extra[:, o, 1] = (cy - cky) * _f(W)
        extra[:, o, 2] = sc0
        extra[:, o, 3] = sc1
        fh = ((ch + one) * _f(HR) - one) * _f(0.5)
        fw = ((cw + one) * _f(WR) - one) * _f(0.5)
        h0 = np.floor(fh)
        w0 = np.floor(fw)
        ah = fh - h0
        aw = fw - w0
        h0i = h0.astype(np.int32)
        w0i = w0.astype(np.int32)
        ridx[:, o] = (np.arange(B, dtype=np.int32)[:, None] * (PADHR * PADHR)
                      + (h0i + 2) * PADHR + (w0i + 2))
        wts[:, o, 0] = (one - ah) * (one - aw)
        wts[:, o, 1] = (one - ah) * aw
        wts[:, o, 2] = ah * (one - aw)
        wts[:, o, 3] = ah * aw

    # host-side bilinear(x) with border clamp (0.03% of the FLOPs)
    fhx = np.clip(((cx + one) * _f(H) - one) * _f(0.5), _f(0.0), _f(H - 1))
    fwx = np.clip(((cy + one) * _f(W) - one) * _f(0.5), _f(0.0), _f(W - 1))
    h0 = np.floor(fhx)
    w0 = np.floor(fwx)
    ah = fhx - h0
    aw = fwx - w0
    h0i = h0.astype(np.int32)
    w0i = w0.astype(np.int32)
    h1i = np.clip(h0i + 1, 0, H - 1)
    w1i = np.clip(w0i + 1, 0, W - 1)
    bi = np.arange(B)[:, None]
    xq = x[:, 0]
    xb = (xq[bi, h0i, w0i] * ((one - ah) * (one - aw))
          + xq[bi, h0i, w1i] * ((one - ah) * aw)
          + xq[bi, h1i, w0i] * (ah * (one - aw))
          + xq[bi, h1i, w1i] * (ah * aw)).astype(np.float32)

    # FU4 block-anchor indices: variant = (row-step, col-step) of the
    # offsets' pixels; exact whenever the steps are in {0,1,2} (always, per
    # the +-1.0 LR spacing; fe_ok guards the fallback regardless).
    ihm, iwm = ihs[:, 0], iws[:, 0]
    rs = ihs[:, 2] - ihm
    cs = iws[:, 3] - iwm
    fe_ok = bool(np.all((rs >= 0) & (rs <= 2) & (cs >= 0) & (cs <= 2))
                 and np.all(ihs[:, 1] == ihm) and np.all(iws[:, 1] == iws[:, 3])
                 and np.all(ihs[:, 3] == ihs[:, 2]) and np.all(iws[:, 2] == iwm))
    fidx4 = (np.arange(B, dtype=np.int32)[:, None] * (9 * H * W)
             + (rs * 3 + cs) * (H * W) + ihm * W + iwm)
    return ridx, fidx, fidx4, fe_ok, wts, extra, xb


class _WPack:
    def __init__(self):
        self.cols = 0
        self.items = []

    def add(self, arr):
        arr = np.asarray(arr, np.float32)
        if arr.ndim == 1:
            arr = arr[:, None]
        assert arr.shape[0] <= 128
        col = self.cols
        self.items.append((col, arr))
        self.cols += arr.shape[1]
        return col

    def build(self, dtype=np.float32):
        nw = (self.cols + 15) // 16 * 16
        out = np.zeros((128, nw), dtype)
        for col, arr in self.items:
            out[: arr.shape[0], col:col + arr.shape[1]] = arr.astype(dtype)
        return out


def _dr_pack(w, r0, rows2, cols):
    # DoubleRow stationary pack: [128, 2*len(cols)] with partition p holding
    # rows (r0+p) then (r0+128+p); rows2 = rows in the second 128-block.
    out = np.zeros((128, 2 * len(cols)), np.float32)
    out[:, 0:len(cols)] = w[r0:r0 + 128][:, cols]
    out[:rows2, len(cols):] = w[r0 + 128:r0 + 128 + rows2][:, cols]
    return out


def _pack_weights(kv, kr, kq):
    """Main (fp32r) pack + optional bf16 pack for the W1 tiles + fp8 pack
    (x F8SC) for the DoubleRow mm2/mm3 weights."""
    wp = _WPack()
    wph = _WPack()
    wp8 = _WPack()
    loc = {}
    w1p = wph if BF16_L1 else wp
    for tag, (w1, b1, w2, b2) in (("v", kv), ("r", kr)):
        if FP8_MM1:
            w1s = w1 * F8SC
            for m in range(2):
                mc = range(m * 128, (m + 1) * 128)
                for p in range(2):
                    loc[f"{tag}w1dr_{p}_{m}"] = wp8.add(
                        _dr_pack(w1s, p * 256, 128, mc))
                loc[f"{tag}w1s_{m}"] = wp8.add(w1s[512:580][:, mc])
        else:
            for c in range(5):
                r0 = c * 128
                for m in range(2):
                    loc[f"{tag}w1_{c}_{m}"] = w1p.add(w1[r0:r0 + KW1[c], m * 128:(m + 1) * 128])
        if FP8_MM23:
            for j in range(5):
                cols = range(j * 128, j * 128 + JW[j])
                loc[f"{tag}w2dr_{j}"] = wp8.add(_dr_pack(w2 * F8SC, 0, 128, cols))
        else:
            for k in range(2):
                for j in range(5):
                    loc[f"{tag}w2_{k}_{j}"] = wp.add(w2[k * 128:(k + 1) * 128, j * 128:j * 128 + JW[j]])
        for m in range(2):
            loc[f"{tag}b1_{m}"] = wp.add(b1[m * 128:(m + 1) * 128])
        b2s = b2 * F8SC if FP8_MM23 else b2
        for j in range(5):
            loc[f"{tag}b2_{j}"] = wp.add(b2s[j * 128:j * 128 + JW[j]])
    qw1, qb1, qw2, qb2 = kq
    if FP8_MM23:
        qw1s = qw1 * F8SC
        for o in range(4):
            for m in range(2):
                mc = range(m * 128, (m + 1) * 128)
                for p in range(2):
                    loc[f"qw1dr_{o}_{p}_{m}"] = wp8.add(
                        _dr_pack(qw1s, o * C9 + p * 256, 128, mc))
                loc[f"qw1s_{o}_{m}"] = wp8.add(qw1s[o * C9 + 512:o * C9 + 576][:, mc])
    else:
        qw1p = wph if (BF16_L1 and QW1_BF) else wp
        for o in range(4):
            for j in range(5):
                r0 = o * C9 + j * 128
                for m in range(2):
                    loc[f"qw1_{o}_{j}_{m}"] = qw1p.add(qw1[r0:r0 + JW[j], m * 128:(m + 1) * 128])
    for m in range(2):
        loc[f"qb1_{m}"] = wp.add(qb1[m * 128:(m + 1) * 128])
        loc[f"qw2_{m}"] = wp.add(qw2[m * 128:(m + 1) * 128, :])
    loc["qb2"] = wp.add(qb2.reshape(1, 1))
    wpk = wp.build(np.float32)
    wpkh = wph.build(ml_dtypes.bfloat16) if BF16_L1 else None
    wpk8 = (wp8.build(ml_dtypes.float8_e4m3)
            if (FP8_MM23 or FP8_MM1) else None)
    return wpk, wpkh, wpk8, loc


# ---------------------------------------------------------------- device IR
_CACHE = {}


def _build(nw, nwh, loc, rep=1, use4=None, bz=False, nw8=0):
    DT1 = BF16 if BF16_L1 else F32      # gather/blend tiles
    DA1 = BF16 if BF16_L1 else F32R     # act (mm1 input) tiles
    DACT = mybir.dt.float8e4 if FP8_MM1 else DA1
    if use4 is None:
        use4 = FEAT4
    # bz: all MLP biases are zero -> the pred gating is a plain
    # TensorTensor mult instead of TensorScalarPtr add+mult.
    WPD = F32

    nc = bacc.Bacc("TRN2", target_bir_lowering=False, debug=False,
                   enable_asserts=False)
    FU = nc.dram_tensor("FU", [B * 9 * H * W, 4 * C9] if use4 else [B * H * W, C9],
                        DT1, kind="ExternalInput").ap()
    RF4 = nc.dram_tensor("RF4", [B * PADHR * PADHR, 4 * C9], DT1,
                         kind="ExternalInput").ap()
    WPK = nc.dram_tensor("WPK", [128, nw], F32R, kind="ExternalInput").ap()
    if BF16_L1 and nwh > 0:
        WPKH = nc.dram_tensor("WPKH", [128, nwh], BF16, kind="ExternalInput").ap()
    F8 = mybir.dt.float8e4
    DR = mybir.MatmulPerfMode.DoubleRow
    if FP8_MM23 or FP8_MM1:
        WPK8 = nc.dram_tensor("WPK8", [128, nw8], F8, kind="ExternalInput").ap()
    IDX = nc.dram_tensor("IDX", [NT, 128, 9], I32, kind="ExternalInput").ap()
    if BLEND_DIAG:
        WTSD = nc.dram_tensor("WTSD", [NT, 4, 128, 512], DT1,
                              kind="ExternalInput").ap()
    else:
        WTS = nc.dram_tensor("WTS", [NT, 128, 16], F32, kind="ExternalInput").ap()
    EXT = nc.dram_tensor("EXT", [NT, 4, 4, 128], DACT, kind="ExternalInput").ap()
    XB = nc.dram_tensor("XB", [NCH, CHQ], F32, kind="ExternalInput").ap()
    OUT = nc.dram_tensor("OUT", [NCH, CHQ], F32, kind="ExternalOutput").ap()

    AF = mybir.ActivationFunctionType
    OP = mybir.AluOpType

    with tile.TileContext(nc) as tc, ExitStack() as ctx:
        wpool = ctx.enter_context(tc.tile_pool(name="w", bufs=1))
        gpo = ctx.enter_context(tc.tile_pool(name="g", bufs=5))
        fpo = ctx.enter_context(tc.tile_pool(name="f", bufs=2))
        sp = ctx.enter_context(tc.tile_pool(name="s", bufs=2))
        ap_ = ctx.enter_context(tc.tile_pool(name="a", bufs=2))
        prp = ctx.enter_context(tc.tile_pool(name="pr", bufs=2))
        pp_t = ctx.enter_context(tc.tile_pool(name="ppt", bufs=1, space="PSUM"))
        pp_h = ctx.enter_context(tc.tile_pool(name="pph", bufs=2, space="PSUM"))
        pp_w = ctx.enter_context(tc.tile_pool(name="ppw", bufs=2, space="PSUM"))
        pp_q = ctx.enter_context(tc.tile_pool(name="ppq", bufs=1, space="PSUM"))

        wsb = wpool.tile([128, nw], F32R)
        nc.sync.dma_start(wsb[:], WPK[:])
        if BF16_L1 and nwh > 0:
            wsbh = wpool.tile([128, nwh], BF16)
            nc.sync.dma_start(wsbh[:], WPKH[:])
        if FP8_MM23 or FP8_MM1:
            wsb8 = wpool.tile([128, nw8], F8)
            nc.sync.dma_start(wsb8[:], WPK8[:])
        ident = wpool.tile([128, 128], DT1)
        make_identity(nc, ident[:])
        # all per-tile gather indices preloaded once: kills the per-section
        # idx-DMA -> indirect-gather dependency chain on the PE critical path
        idx_sb = wpool.tile([128, NT, 9], I32)
        nc.sync.dma_start(idx_sb[:], IDX.rearrange("t p c -> p t c"))

        def wap(key, rows, width):
            c = loc[key]
            return wsb[0:rows, c:c + width]

        def wap1(key, rows, width):
            c = loc[key]
            return (wsbh if BF16_L1 else wsb)[0:rows, c:c + width]

        def wap8(key, rows, width):
            c = loc[key]
            return wsb8[0:rows, c:c + width]

        PRD = BF16 if (BF16_L1 and QW1_BF) else F32R
        wq1ap = wap1 if (BF16_L1 and QW1_BF) else wap
        TQ = CHQ // 128
        NCHR = NCH * rep
        act4s = {}
        qhs = {}

        def act_sl(cn, o, br, c, r0, r1, c0, c1):
            base = (br * 5 + c) * CHQ
            return act4s[cn][o][r0:r1, base + c0:base + c1]

        def emit_tsec(cn, t):
            """Gather + blend + transpose + copy-back for 128-query tile t
            of chunk cn, all 4 offsets."""
            if cn not in act4s:
                # merged activation tile per offset: K chunks at cols
                # c*CHQ (c=0..4), R chunks at cols (5+c)*CHQ
                act4s[cn] = [ap_.tile([128, 10 * CHQ], DACT, tag=f"a{o}",
                                      name=f"a{o}") for o in range(4)]
            ti = (cn % NCH) * TQ + t
            it = idx_sb[:, ti, :]
            if not BLEND_DIAG:
                wt = sp.tile([128, 16], F32, tag="wts")
                nc.sync.dma_start(wt[:], WTS[ti])
            if use4:
                ftq = fpo.tile([128, 4 * C9], DT1, tag="ftq")
                nc.gpsimd.indirect_dma_start(
                    out=ftq[:], out_offset=None, in_=FU[:],
                    in_offset=bass.IndirectOffsetOnAxis(ap=it[:, 8:9],
                                                        axis=0))
            if BLEND_DIAG:
                wds = []
                for o in range(4):
                    wd = sp.tile([128, 512], DT1, tag=f"wd{o}",
                                 name=f"wd{o}")
                    nc.sync.dma_start(wd[:], WTSD[ti, o])
                    wds.append(wd)
            for o in range(4):
                if use4:
                    ft = ftq[:, o * C9:(o + 1) * C9]
                else:
                    ft_t = fpo.tile([128, C9], DT1, tag="ft")
                    nc.gpsimd.indirect_dma_start(
                        out=ft_t[:], out_offset=None, in_=FU[:],
                        in_offset=bass.IndirectOffsetOnAxis(
                            ap=it[:, 4 + o:5 + o], axis=0))
                    ft = ft_t[:]
                g = gpo.tile([128, 4 * C9], DT1, tag="gt")
                nc.gpsimd.indirect_dma_start(
                    out=g[:], out_offset=None, in_=RF4[:],
                    in_offset=bass.IndirectOffsetOnAxis(ap=it[:, o:o + 1],
                                                        axis=0))

                def seg(p):
                    return g[:, p * C9:(p + 1) * C9]

                tpF = pp_t.tile([128, 640], DT1, tag="tpF")
                tpR = pp_t.tile([128, 640], DT1, tag="tpR")
                if BLEND_DIAG:
                    # blend fused into the transposes: 4 accumulating
                    # scale-transposes per chunk with per-query diag(weight)
                    # as the rhs. No elementwise blend ops at all.
                    wd = wds[o]
                    for c in range(5):
                        fw_ = JW[c]
                        nc.tensor.transpose(
                            tpF[:fw_, c * 128:(c + 1) * 128],
                            ft[:, c * 128:c * 128 + fw_], ident[:])
                        for k in range(4):
                            nc.tensor.matmul(
                                tpR[:fw_, c * 128:(c + 1) * 128],
                                g[:, k * C9 + c * 128:k * C9 + c * 128 + fw_],
                                wd[:, k * 128:(k + 1) * 128],
                                is_transpose=True, start=(k == 0),
                                stop=(k == 3))
                else:
                    wv = [wt[:, o * 4 + k:o * 4 + k + 1] for k in range(4)]
                    tmp0 = sp.tile([128, C9], DT1, tag="tmp0")
                    tmp1 = sp.tile([128, C9], DT1, tag="tmp1")
                    acc0 = sp.tile([128, C9], DT1, tag="acc0")
                    blend_eng = nc.gpsimd if BLEND_POOL else nc.vector
                    nc.scalar.activation(tmp0[:], seg(0), AF.Copy, scale=wv[0])
                    blend_eng.scalar_tensor_tensor(
                        out=acc0[:], in0=seg(1), scalar=wv[1], in1=tmp0[:],
                        op0=OP.mult, op1=OP.add)
                    nc.scalar.activation(tmp1[:], seg(2), AF.Copy, scale=wv[2])
                    blend_eng.scalar_tensor_tensor(
                        out=tmp1[:], in0=seg(3), scalar=wv[3], in1=tmp1[:],
                        op0=OP.mult, op1=OP.add)

                    for c in range(5):
                        fw_ = JW[c]
                        nc.tensor.transpose(
                            tpF[:fw_, c * 128:(c + 1) * 128],
                            ft[:, c * 128:c * 128 + fw_], ident[:])
                        nc.tensor.matmul(
                            tpR[:fw_, c * 128:(c + 1) * 128],
                            acc0[:, c * 128:c * 128 + fw_], ident[:],
                            is_transpose=True, start=True, stop=False)
                        nc.tensor.matmul(
                            tpR[:fw_, c * 128:(c + 1) * 128],
                            tmp1[:, c * 128:c * 128 + fw_], ident[:],
                            is_transpose=True, start=False, stop=True)
                for br, tp in ((0, tpF), (1, tpR)):
                    dst = act4s[cn][o][0:128,
                                       br * 5 * CHQ:(br + 1) * 5 * CHQ]
                    dst = dst.rearrange("p (c q) -> p c q", c=5)
                    dst = dst[:, 0:4, t * 128:(t + 1) * 128]
                    src = tp[0:128, 0:512].rearrange("p (c q) -> p c q",
                                                     c=4)
                    if CPB_DMA:
                        nc.sync.dma_start(dst, src)
                        nc.sync.dma_start(
                            act_sl(cn, o, br, 4, 0, 64,
                                   t * 128, (t + 1) * 128),
                            tp[0:64, 512:640])
                    else:
                        nc.vector.tensor_copy(out=dst, in_=src)
                        nc.scalar.copy(
                            act_sl(cn, o, br, 4, 0, 64,
                                   t * 128, (t + 1) * 128),
                            tp[0:64, 512:640])
                nc.sync.dma_start(act_sl(cn, o, 0, 4, 64, 68,
                                         t * 128, (t + 1) * 128),
                                  EXT[ti, o])
                nc.sync.dma_start(act_sl(cn, o, 1, 4, 64, 68,
                                         t * 128, (t + 1) * 128),
                                  EXT[ti, o])

        def emit_mlp(cn, o, br):
            """v/r MLP + gating + final-MLP accumulation for one
            (offset, branch) of chunk cn."""
            if cn not in qhs:
                qhs[cn] = [pp_q.tile([128, CHQ], F32, tag=f"qh{m}",
                                     name=f"qh{m}") for m in range(2)]
            qh_ps = qhs[cn]
            tag = "v" if br == 0 else "r"
            first = (o == 0 and br == 0)
            last_ob = (o == 3 and br == 1)
            if FP8_MM23:
                h8 = sp.tile([128, 2 * CHQ], F8, tag="h")
                for m in range(2):
                    hp = pp_h.tile([128, CHQ], F32, tag="hp")
                    if FP8_MM1:
                        base = (br * 5) * CHQ
                        a2 = act4s[cn][o]
                        for p in range(2):
                            nc.tensor.matmul(
                                hp[:],
                                wap8(f"{tag}w1dr_{p}_{m}", 128,
                                     256).rearrange("p (k j) -> p k j", k=2),
                                a2[0:128,
                                   base + 2 * p * CHQ:base + 2 * (p + 1) * CHQ
                                   ].rearrange("p (k q) -> p k q", k=2),
                                start=(p == 0), stop=False, perf_mode=DR)
                        nc.tensor.matmul(
                            hp[:], wap8(f"{tag}w1s_{m}", 68, 128),
                            act_sl(cn, o, br, 4, 0, 68, 0, CHQ),
                            start=False, stop=True)
                    else:
                        for c in range(5):
                            kw = KW1[c]
                            rhs = act_sl(cn, o, br, c, 0, kw, 0, CHQ)
                            nc.tensor.matmul(
                                hp[:], wap1(f"{tag}w1_{c}_{m}", kw, 128),
                                rhs, start=(c == 0), stop=(c == 4))
                    nc.scalar.activation(h8[:, m * CHQ:(m + 1) * CHQ],
                                         hp[:], AF.Relu,
                                         bias=wap(f"{tag}b1_{m}", 128, 1),
                                         scale=(1.0 / F8SC if FP8_MM1
                                                else 1.0))
                pr = prp.tile([128, 5 * CHQ], F8, tag=f"pr{tag}",
                              name=f"pr{tag}")
                h2 = h8[:].rearrange("p (k q) -> p k q", k=2)
                for j in range(5):
                    jw = JW[j]
                    wp_ = pp_w.tile([128, CHQ], WPD, tag="wp")
                    nc.tensor.matmul(
                        wp_[:jw, :],
                        wap8(f"{tag}w2dr_{j}", 128,
                             2 * jw).rearrange("p (k j) -> p k j", k=2),
                        h2, start=True, stop=True, perf_mode=DR)
                    prj = pr[0:jw, j * CHQ:(j + 1) * CHQ]
                    if bz:
                        nc.vector.tensor_tensor(
                            out=prj, in0=wp_[:jw, :],
                            in1=act_sl(cn, o, br, j, 0, jw, 0, CHQ),
                            op=OP.mult)
                    else:
                        nc.vector.scalar_tensor_tensor(
                            out=prj, in0=wp_[:jw, :],
                            scalar=wap(f"{tag}b2_{j}", jw, 1),
                            in1=act_sl(cn, o, br, j, 0, jw, 0, CHQ),
                            op0=OP.add, op1=OP.mult)
                for m in range(2):
                    for p in range(2):
                        nc.tensor.matmul(
                            qh_ps[m][:],
                            wap8(f"qw1dr_{o}_{p}_{m}", 128,
                                 256).rearrange("p (k j) -> p k j", k=2),
                            pr[:, 2 * p * CHQ:2 * (p + 1) * CHQ].rearrange(
                                "p (k q) -> p k q", k=2),
                            start=(first and p == 0), stop=False,
                            perf_mode=DR)
                    nc.tensor.matmul(
                        qh_ps[m][:], wap8(f"qw1s_{o}_{m}", 64, 128),
                        pr[0:64, 4 * CHQ:5 * CHQ],
                        start=False, stop=last_ob)
                return
            hs = []
            for m in range(2):
                hp = pp_h.tile([128, CHQ], F32, tag="hp")
                for c in range(5):
                    kw = KW1[c]
                    rhs = act_sl(cn, o, br, c, 0, kw, 0, CHQ)
                    nc.tensor.matmul(
                        hp[:], wap1(f"{tag}w1_{c}_{m}", kw, 128),
                        rhs, start=(c == 0), stop=(c == 4))
                h = sp.tile([128, CHQ], F32R, tag="h")
                nc.scalar.activation(h[:], hp[:], AF.Relu,
                                     bias=wap(f"{tag}b1_{m}", 128, 1))
                hs.append(h)
            for j in range(5):
                jw = JW[j]
                wp_ = pp_w.tile([128, CHQ], WPD, tag="wp")
                for k in range(2):
                    nc.tensor.matmul(
                        wp_[:jw, :], wap(f"{tag}w2_{k}_{j}", 128, jw),
                        hs[k][:], start=(k == 0), stop=(k == 1))
                pr = prp.tile([128, CHQ], PRD, tag=f"pr{tag}",
                              name=f"pr{tag}")
                if bz:
                    nc.vector.tensor_tensor(
                        out=pr[:jw, :], in0=wp_[:jw, :],
                        in1=act_sl(cn, o, br, j, 0, jw, 0, CHQ),
                        op=OP.mult)
                else:
                    nc.vector.scalar_tensor_tensor(
                        out=pr[:jw, :], in0=wp_[:jw, :],
                        scalar=wap(f"{tag}b2_{j}", jw, 1),
                        in1=act_sl(cn, o, br, j, 0, jw, 0, CHQ),
                        op0=OP.add, op1=OP.mult)
                for m in range(2):
                    nc.tensor.matmul(
                        qh_ps[m][:], wq1ap(f"qw1_{o}_{j}_{m}", jw, 128),
                        pr[:jw, :], start=(first and j == 0),
                        stop=(last_ob and j == 4))

        def emit_tail(cn):
            qh_ps = qhs.pop(cn)
            act4s.pop(cn)
            qsc = 1.0 / (F8SC * F8SC) if FP8_MM23 else 1.0
            qouts = pp_h.tile([128, CHQ], F32, tag="hp")
            for m in range(2):
                qh = sp.tile([128, CHQ], F32R, tag="qh_sb")
                nc.scalar.activation(qh[:], qh_ps[m][:], AF.Relu,
                                     bias=wap(f"qb1_{m}", 128, 1),
                                     scale=qsc)
                nc.tensor.matmul(qouts[0:1, :], wap(f"qw2_{m}", 128, 1),
                                 qh[:], start=(m == 0), stop=(m == 1))
            chn = cn % NCH
            xbt = sp.tile([1, CHQ], F32, tag="xbt")
            nc.sync.dma_start(xbt[:], XB[chn:chn + 1, :])
            osb = sp.tile([1, CHQ], F32, tag="osb")
            nc.vector.scalar_tensor_tensor(
                out=osb[:], in0=qouts[0:1, :], scalar=wap("qb2", 1, 1),
                in1=xbt[:], op0=OP.add, op1=OP.add)
            nc.sync.dma_start(OUT[chn:chn + 1, :], osb[:])

        # software pipeline: interleave chunk cn's MLP sections with chunk
        # cn+1's gather/transpose sections so every engine's in-order stream
        # always has both kinds of work available.
        for t in range(TQ):
            emit_tsec(0, t)
        for cn in range(NCHR):
            nxt = cn + 1
            for o in range(4):
                emit_mlp(cn, o, 0)
                emit_mlp(cn, o, 1)
                if nxt < NCHR:
                    emit_tsec(nxt, o if TQ == 4 else o % TQ)
            emit_tail(cn)

    nc.compile()
    return nc


def _prepare(x, feature, ref_feat_hr_res, coord, cell, kv, kr, kq):
    """Returns (nw, nwh, nw8, loc, in_maps, fe_ok)."""
    ridx, fidx, fidx4, fe_ok, wts, extra, xb = _tables(coord, cell, x)
    use4 = FEAT4 and fe_ok
    FU = _feat_table4(feature) if use4 else _unfold_table(feature)
    RF4 = _ref_table4(ref_feat_hr_res)
    WPK, WPKH, WPK8, loc = _pack_weights(kv, kr, kq)

    if BF16_L1:
        FU = FU.astype(ml_dtypes.bfloat16)
        RF4 = RF4.astype(ml_dtypes.bfloat16)
    ext_np = (ml_dtypes.float8_e4m3 if FP8_MM1
              else ml_dtypes.bfloat16 if BF16_L1 else np.float32)
    wtsd_np = ml_dtypes.bfloat16 if BF16_L1 else np.float32

    ar128 = np.arange(128)
    in_maps = []
    for core in range(NCORES):
        qs = slice(core * QSH, (core + 1) * QSH)
        idx_t = np.zeros((NT, 128, 9), np.int32)
        wts_t = np.empty((NT, 128, 16), np.float32)
        ext_t = np.empty((NT, 4, 4, 128), np.float32)
        for b in range(B):
            rr = ridx[b, :, qs].reshape(4, 16, 128)
            ff = fidx[b, :, qs].reshape(4, 16, 128)
            f4 = fidx4[b, qs].reshape(16, 128)
            ww = wts[b, :, :, qs].reshape(4, 4, 16, 128)
            ee = extra[b, :, :, qs].reshape(4, 4, 16, 128)
            for t16 in range(16):
                t = b * 16 + t16
                idx_t[t, :, 0:4] = rr[:, t16, :].T
                idx_t[t, :, 4:8] = ff[:, t16, :].T
                idx_t[t, :, 8] = f4[t16, :]
                wts_t[t] = ww[:, :, t16, :].reshape(16, 128).T
                ext_t[t] = ee[:, :, t16, :]
        xb_t = np.ascontiguousarray(xb[:, qs].reshape(NCH, CHQ))
        m = {"FU": FU, "RF4": RF4, "WPK": WPK, "IDX": idx_t,
             "EXT": ext_t.astype(ext_np), "XB": xb_t}
        if BLEND_DIAG:
            wtsd = np.zeros((NT, 4, 128, 4, 128), wtsd_np)
            for o in range(4):
                for c in range(4):
                    wtsd[:, o, ar128, c, ar128] = wts_t[:, :, o * 4 + c].astype(wtsd_np)
            m["WTSD"] = wtsd.reshape(NT, 4, 128, 512)
        else:
            m["WTS"] = wts_t
        if BF16_L1 and WPKH.shape[1] > 0:
            m["WPKH"] = WPKH
        if FP8_MM23:
            m["WPK8"] = WPK8
        in_maps.append(m)
    nwh = WPKH.shape[1] if WPKH is not None else 0
    nw8 = WPK8.shape[1] if WPK8 is not None else 0
    return WPK.shape[1], nwh, nw8, loc, in_maps, fe_ok


def kernel(x, feature, ref_feat_hr_res, coord, cell,
           k_w1, k_b1, k_w2, k_b2, v_w1, v_b1, v_w2, v_b2,
           r_w1, r_b1, r_w2, r_b2, q_w1, q_b1, q_w2, q_b2):
    x = np.asarray(x, np.float32)
    feature = np.asarray(feature, np.float32)
    ref_feat_hr_res = np.asarray(ref_feat_hr_res, np.float32)
    coord = np.asarray(coord, np.float32)
    cell = np.asarray(cell, np.float32)
    asf = lambda *a: tuple(np.asarray(v, np.float32) for v in a)

    nw, nwh, nw8, loc, in_maps, fe_ok = _prepare(
        x, feature, ref_feat_hr_res, coord, cell,
        asf(v_w1, v_b1, v_w2, v_b2),
        asf(r_w1, r_b1, r_w2, r_b2),
        asf(q_w1, q_b1, q_w2, q_b2))
    use4 = FEAT4 and fe_ok
    bz = all(not np.any(np.asarray(b)) for b in
             (v_b1, v_b2, r_b1, r_b2, q_b1, q_b2))

    key = ("k4", nw, nwh, nw8, BF16_L1, QW1_BF, FP8_MM23, FP8_MM1, CHQ,
           use4, bz)
    if key not in _CACHE:
        _CACHE[key] = _build(nw, nwh, loc, use4=use4, bz=bz, nw8=nw8)
    nc = _CACHE[key]
    kernel._build_args = (nw, nwh, loc)
    kernel._build_kw = {"nw8": nw8}
    kernel._use4 = use4
    kernel._bz = bz

    res = run_bass_kernel_spmd(nc, in_maps, core_ids=list(range(NCORES)))
    kernel._last = (nc, in_maps)

    out = np.empty((B, Q, 1), np.float32)
    for core in range(NCORES):
        o = res.results[core]["OUT"].reshape(B, QSH)
        out[:, core * QSH:(core + 1) * QSH, 0] = o
    return out

